# revision 2
# baseline (speedup 1.0000x reference)
"""Multi-head attention (B=4, S=1024, D=1024, H=16) on 8 TRN2 NeuronCores.

Sharding: data parallel on batch (4) x tensor parallel on heads (2 groups of
8 heads).  Core c handles batch c//2 and heads (c%2)*8 .. (c%2)*8+7.

Per-core dataflow (everything in "transposed" space so no on-device
transposes are needed):
  QT [512,1024] = WqT_loc.T-contracted projection  (d_out on partitions)
  KT [512,1024] likewise
  V  [1024,512] natural layout (k-positions on partitions)
  V2 [k,h,65]   = [V*mask | mask]  (extra column -> masked row-sums via matmul)
  scoresT[k,q]  = KT_h.T @ QT_h    (K=64 contraction, head pairs row-packed)
  pT = exp(scoresT/8)              (no max subtraction; scores are O(1))
  attnV: psum[0:65] = V2_h.T @ pT  (rows 0:64 = numerator^T, row 64 = denom)
  normalize via reciprocal + ones-matmul partition broadcast
  Wo: out[q,o] partial = CT.T @ WoT_loc ; host adds the two head-group halves.

Matmuls run as float32r (~1.5e-4 rel err, full PE rate at N=512).
"""
import sys

if '/opt/trn_rl_repo' not in sys.path:
    sys.path.insert(0, '/opt/trn_rl_repo')

import numpy as np

P = 128
B, S, D = 4, 1024, 1024
DL = 512          # local d_out (8 heads x 64)
H = 8             # local heads
E = 64            # head dim
IC = D // P       # 8 contraction chunks for projections
KC = S // P       # 8 key-position chunks
T4 = DL // P      # 4 tiles holding QT/KT/CT rows
NQ = 512          # matmul moving free dim
N_CORES = 8

_prog_cache = {}


def build_program():
    import concourse.tile as tile
    from concourse import bacc, mybir

    F32 = mybir.dt.float32
    F32R = mybir.dt.float32r
    EXP = mybir.ActivationFunctionType.Exp
    MULT = mybir.AluOpType.mult

    nc = bacc.Bacc("TRN2", target_bir_lowering=False, debug=False,
                   enable_asserts=False, num_devices=N_CORES)

    xtq = nc.dram_tensor("xtq", (D, S), F32R, kind="ExternalInput").ap()
    xtk = nc.dram_tensor("xtk", (D, S), F32R, kind="ExternalInput").ap()
    xtv = nc.dram_tensor("xtv", (D, S), F32R, kind="ExternalInput").ap()
    wq = nc.dram_tensor("wq", (D, DL), F32R, kind="ExternalInput").ap()
    wk = nc.dram_tensor("wk", (D, DL), F32R, kind="ExternalInput").ap()
    wv = nc.dram_tensor("wv", (D, DL), F32R, kind="ExternalInput").ap()
    wo = nc.dram_tensor("wo", (DL, D), F32R, kind="ExternalInput").ap()
    maskd = nc.dram_tensor("maskd", (P, KC), F32, kind="ExternalInput").ap()
    out = nc.dram_tensor("out", (S, D), F32, kind="ExternalOutput").ap()

    xtq_c = xtq.rearrange("(ic p) s -> ic p s", p=P)
    xtk_c = xtk.rearrange("(ic p) s -> ic p s", p=P)
    xtv_c = xtv.rearrange("(ic p) s -> ic p s", p=P)
    wq_c = wq.rearrange("(ic p) o -> ic p o", p=P)
    wk_c = wk.rearrange("(ic p) o -> ic p o", p=P)
    wv_c = wv.rearrange("(ic p) o -> ic p o", p=P)
    wo_c = wo.rearrange("(t p) o -> t p o", p=P)

    with tile.TileContext(nc) as tc:
        with tc.tile_pool(name="xt", bufs=12) as xt_pool, \
             tc.tile_pool(name="wp", bufs=10) as w_pool, \
             tc.tile_pool(name="wop", bufs=4) as wo_pool, \
             tc.tile_pool(name="qk", bufs=8) as qk_pool, \
             tc.tile_pool(name="v2p", bufs=8) as v2_pool, \
             tc.tile_pool(name="pp", bufs=12) as p_pool, \
             tc.tile_pool(name="ctp", bufs=4) as ct_pool, \
             tc.tile_pool(name="sm", bufs=4) as small, \
             tc.tile_pool(name="ob", bufs=3) as out_pool, \
             tc.tile_pool(name="psP", bufs=2, space="PSUM") as psP, \
             tc.tile_pool(name="psS", bufs=3, space="PSUM") as psS, \
             tc.tile_pool(name="psO", bufs=2, space="PSUM") as psO, \
             tc.tile_pool(name="psR", bufs=1, space="PSUM") as psR:

            # ---- constants / small inputs ----
            mask_sb = small.tile([P, KC], F32, tag="mask")
            nc.sync.dma_start(mask_sb[:], maskd[:])
            ones_f = small.tile([1, E], F32, tag="ones_f")
            nc.gpsimd.memset(ones_f[:], 1.0)
            ones_r = small.tile([1, E], F32R, tag="ones_r")
            nc.vector.tensor_copy(ones_r[:], ones_f[:])

            # ---- Wo weights (needed last; load early, DMA is idle later) ----
            wo_sb = []
            for t in range(T4):
                wt = wo_pool.tile([P, D], F32R, tag="wo", name=f"wo{t}")
                nc.sync.dma_start(wt[:], wo_c[t])
                wo_sb.append(wt)

            # ---- projections ----
            def load_chunks(dram_c, n, tag, pool, shape):
                tiles = []
                for ic in range(n):
                    t = pool.tile(shape, F32R, tag=tag, name=f"{tag}{ic}")
                    nc.sync.dma_start(t[:], dram_c[ic])
                    tiles.append(t)
                return tiles

            qt = [qk_pool.tile([P, S], F32R, tag="qk", name=f"qt{i}") for i in range(T4)]
            kt = [qk_pool.tile([P, S], F32R, tag="qk", name=f"kt{i}") for i in range(T4)]

            # Q^T and K^T: psum[o_chunk 128, s 512] = sum_ic w[ic,:,ochunk].T @ xt[ic,:,schunk]
            for name, x_c, w_c, dest in (("q", xtq_c, wq_c, qt),
                                         ("k", xtk_c, wk_c, kt)):
                x_sb = load_chunks(x_c, IC, "xt", xt_pool, [P, S])
                w_sb = load_chunks(w_c, IC, "wp", w_pool, [P, DL])
                for t in range(T4):
                    for sc in range(2):
                        ps = psP.tile([P, NQ], F32, tag="psP")
                        for ic in range(IC):
                            nc.tensor.matmul(
                                ps[:],
                                w_sb[ic][:, t * P:(t + 1) * P],
                                x_sb[ic][:, sc * NQ:(sc + 1) * NQ],
                                start=(ic == 0), stop=(ic == IC - 1))
                        nc.scalar.copy(dest[t][:, sc * NQ:(sc + 1) * NQ], ps[:])

            # V: psum[s_chunk 128, o 512] = sum_ic xtv[ic,:,schunk].T @ wv[ic]
            x_sb = load_chunks(xtv_c, IC, "xt", xt_pool, [P, S])
            w_sb = load_chunks(wv_c, IC, "wp", w_pool, [P, DL])
            v2 = []
            for skc in range(KC):
                ps = psP.tile([P, NQ], F32, tag="psP")
                for ic in range(IC):
                    nc.tensor.matmul(
                        ps[:],
                        x_sb[ic][:, skc * P:(skc + 1) * P],
                        w_sb[ic][:],
                        start=(ic == 0), stop=(ic == IC - 1))
                v2t = v2_pool.tile([P, H, E + 1], F32R, tag="v2")
                nc.vector.tensor_scalar_mul(
                    v2t[:, :, 0:E],
                    ps[:].rearrange("p (h e) -> p h e", h=H),
                    mask_sb[:, skc:skc + 1])
                nc.vector.tensor_copy(
                    v2t[:, :, E:E + 1],
                    mask_sb[:, skc:skc + 1, None].to_broadcast((P, H, 1)))
                v2.append(v2t)

            # ---- attention (software-pipelined by one (h, qc) step) ----
            ct = [ct_pool.tile([P, S], F32R, tag="ct", name=f"ct{i}") for i in range(T4)]

            def emit_attnv(h, qc, p_tiles):
                pso = psO.tile([P, NQ], F32, tag="psO")
                for kc in range(KC):
                    nc.tensor.matmul(
                        pso[0:E + 1, :],
                        v2[kc][:, h, :],
                        p_tiles[kc][:],
                        start=(kc == 0), stop=(kc == KC - 1))
                recip_f = small.tile([1, NQ], F32, tag="recip_f")
                nc.vector.reciprocal(recip_f[:], pso[E:E + 1, :])
                recip_r = small.tile([1, NQ], F32R, tag="recip_r")
                nc.vector.tensor_copy(recip_r[:], recip_f[:])
                psr = psR.tile([P, NQ], F32, tag="psR")
                nc.tensor.matmul(psr[0:E, :], ones_r[:], recip_r[:],
                                 start=True, stop=True)
                rb = small.tile([E, NQ], F32, tag="rb")
                nc.scalar.copy(rb[:], psr[0:E, :])
                nc.vector.tensor_tensor(
                    ct[h // 2][(h % 2) * E:(h % 2) * E + E,
                               qc * NQ:(qc + 1) * NQ],
                    pso[0:E, :], rb[:], op=MULT)

            pending = None
            for h in range(H):
                t, half = h // 2, h % 2
                pb = half * E
                for qc in range(2):
                    p_tiles = []
                    for kc in range(KC):
                        pss = psS.tile([P, NQ], F32, tag="psS")
                        nc.tensor.matmul(
                            pss[:],
                            kt[t][pb:pb + E, kc * P:(kc + 1) * P],
                            qt[t][pb:pb + E, qc * NQ:(qc + 1) * NQ],
                            start=True, stop=True,
                            tile_position=(pb, 0))
                        pt = p_pool.tile([P, NQ], F32R, tag="pt")
                        nc.scalar.activation(pt[:], pss[:], EXP, scale=0.125)
                        p_tiles.append(pt)
                    if pending is not None:
                        emit_attnv(*pending)
                    pending = (h, qc, p_tiles)
            emit_attnv(*pending)

            # ---- output projection: out[q,o] = sum_t ct[t].T @ wo[t] ----
            for qc8 in range(KC):
                for oc in range(2):
                    ps = psP.tile([P, NQ], F32, tag="psP")
                    for t in range(T4):
                        nc.tensor.matmul(
                            ps[:],
                            ct[t][:, qc8 * P:(qc8 + 1) * P],
                            wo_sb[t][:, oc * NQ:(oc + 1) * NQ],
                            start=(t == 0), stop=(t == T4 - 1))
                    osb = out_pool.tile([P, NQ], F32, tag="osb")
                    nc.vector.tensor_copy(osb[:], ps[:])
                    nc.sync.dma_start(
                        out[qc8 * P:(qc8 + 1) * P, oc * NQ:(oc + 1) * NQ],
                        osb[:])

    nc.compile()
    return nc


def make_in_maps(queries, keys, values, valid_lens, W_q, W_k, W_v, W_o):
    queries = np.asarray(queries, dtype=np.float32)
    keys = np.asarray(keys, dtype=np.float32)
    values = np.asarray(values, dtype=np.float32)
    valid_lens = np.asarray(valid_lens)
    W_q = np.asarray(W_q, dtype=np.float32)
    W_k = np.asarray(W_k, dtype=np.float32)
    W_v = np.asarray(W_v, dtype=np.float32)
    W_o = np.asarray(W_o, dtype=np.float32)

    xtq = [np.ascontiguousarray(queries[b].T) for b in range(B)]
    xtk = [np.ascontiguousarray(keys[b].T) for b in range(B)]
    xtv = [np.ascontiguousarray(values[b].T) for b in range(B)]
    wqt = [np.ascontiguousarray(W_q[hg * DL:(hg + 1) * DL, :].T) for hg in range(2)]
    wkt = [np.ascontiguousarray(W_k[hg * DL:(hg + 1) * DL, :].T) for hg in range(2)]
    wvt = [np.ascontiguousarray(W_v[hg * DL:(hg + 1) * DL, :].T) for hg in range(2)]
    wot = [np.ascontiguousarray(W_o[:, hg * DL:(hg + 1) * DL].T) for hg in range(2)]

    in_maps = []
    for c in range(N_CORES):
        b, hg = c // 2, c % 2
        L = int(valid_lens[b])
        k_idx = np.arange(S).reshape(KC, P).T  # [P, KC]
        maskd = (k_idx < L).astype(np.float32)
        in_maps.append({
            "xtq": xtq[b], "xtk": xtk[b], "xtv": xtv[b],
            "wq": wqt[hg], "wk": wkt[hg], "wv": wvt[hg], "wo": wot[hg],
            "maskd": np.ascontiguousarray(maskd),
        })
    return in_maps


def gather(results):
    out = np.empty((B, S, D), dtype=np.float32)
    for b in range(B):
        out[b] = results[2 * b]["out"] + results[2 * b + 1]["out"]
    return out


def kernel(queries, keys, values, valid_lens, W_q, W_k, W_v, W_o):
    from concourse.bass_utils import run_bass_kernel_spmd

    if "nc" not in _prog_cache:
        _prog_cache["nc"] = build_program()
    nc = _prog_cache["nc"]

    in_maps = make_in_maps(queries, keys, values, valid_lens,
                           W_q, W_k, W_v, W_o)
    res = run_bass_kernel_spmd(nc, in_maps, core_ids=list(range(N_CORES)))
    return gather(res.results)


# revision 7
# speedup vs baseline: 1.0550x; 1.0550x over previous
"""Multi-head attention (B=4, S=1024, D=1024, H=16) on 8 TRN2 NeuronCores.

Sharding: data parallel on batch (4) x tensor parallel on heads (2 groups of
8 heads).  Core c handles batch c//2 and heads (c%2)*8 .. (c%2)*8+7.

Per-core dataflow (everything in "transposed" space so no on-device
transposes are needed):
  QT [512,1024] (d_out on partitions), KT likewise, V [1024,512] natural.
  V2 [k,h,65] = [V*mask | mask]  (65th column -> masked row-sums via matmul)
  scoresT[k,q] = KT_h.T @ QT_h   (K=64 contraction)
  pT = exp(scoresT/8)            (no max subtraction; scores are O(1))
  attnV psum[0:65] = V2_h.T @ pT (rows 0:64 numerator^T, row 64 denominator)
  normalize: fast reciprocal + ones-matmul partition broadcast (3-stage
  software pipeline keeps it off the PE critical path)
  Wo: out[q,o] partial = CT.T @ WoT_loc ; host adds the two head-group halves.

Matmuls run as float32r (~1.5e-4 rel err, full PE rate at N=512).
"""
import sys

if '/opt/trn_rl_repo' not in sys.path:
    sys.path.insert(0, '/opt/trn_rl_repo')

import numpy as np

P = 128
B, S, D = 4, 1024, 1024
DL = 512          # local d_out (8 heads x 64)
H = 8             # local heads
E = 64            # head dim
IC = D // P       # 8 contraction chunks for projections
KC = S // P       # 8 key-position chunks
T4 = DL // P      # 4 tiles holding QT/KT/CT rows
NQ = 512          # matmul moving free dim
N_CORES = 8

_prog_cache = {}


def build_program(paired_exp=True, dve_evac=True, recip_mode='accurate'):
    import concourse.tile as tile
    from concourse import bacc, mybir

    F32 = mybir.dt.float32
    F32R = mybir.dt.float32r
    EXP = mybir.ActivationFunctionType.Exp
    MULT = mybir.AluOpType.mult

    nc = bacc.Bacc("TRN2", target_bir_lowering=False, debug=False,
                   enable_asserts=False, num_devices=N_CORES)

    xtq = nc.dram_tensor("xtq", (D, S), F32R, kind="ExternalInput").ap()
    xtk = nc.dram_tensor("xtk", (D, S), F32R, kind="ExternalInput").ap()
    xtv = nc.dram_tensor("xtv", (D, S), F32R, kind="ExternalInput").ap()
    wq = nc.dram_tensor("wq", (D, DL), F32R, kind="ExternalInput").ap()
    wk = nc.dram_tensor("wk", (D, DL), F32R, kind="ExternalInput").ap()
    wv = nc.dram_tensor("wv", (D, DL), F32R, kind="ExternalInput").ap()
    wo = nc.dram_tensor("wo", (DL, D), F32R, kind="ExternalInput").ap()
    maskd = nc.dram_tensor("maskd", (P, KC), F32, kind="ExternalInput").ap()
    out = nc.dram_tensor("out", (S, D), F32, kind="ExternalOutput").ap()

    xtq_c = xtq.rearrange("(ic p) s -> ic p s", p=P)
    xtk_c = xtk.rearrange("(ic p) s -> ic p s", p=P)
    xtv_c = xtv.rearrange("(ic p) s -> ic p s", p=P)
    wq_c = wq.rearrange("(ic p) o -> ic p o", p=P)
    wk_c = wk.rearrange("(ic p) o -> ic p o", p=P)
    wv_c = wv.rearrange("(ic p) o -> ic p o", p=P)
    wo_c = wo.rearrange("(t p) o -> t p o", p=P)

    with tile.TileContext(nc) as tc:
        with tc.tile_pool(name="xt", bufs=8) as xt_pool, \
             tc.tile_pool(name="wp", bufs=8) as w_pool, \
             tc.tile_pool(name="wop", bufs=4) as wo_pool, \
             tc.tile_pool(name="qk", bufs=8) as qk_pool, \
             tc.tile_pool(name="v2p", bufs=8) as v2_pool, \
             tc.tile_pool(name="pp", bufs=10) as p_pool, \
             tc.tile_pool(name="ctp", bufs=4) as ct_pool, \
             tc.tile_pool(name="sm", bufs=2) as small, \
             tc.tile_pool(name="rbp", bufs=2) as rb_pool, \
             tc.tile_pool(name="ob", bufs=2) as out_pool, \
             tc.tile_pool(name="psP", bufs=2, space="PSUM") as psP, \
             tc.tile_pool(name="psS", bufs=2, space="PSUM") as psS, \
             tc.tile_pool(name="psO", bufs=2, space="PSUM") as psO:

            # ---- constants / small inputs ----
            mask_sb = small.tile([P, KC], F32, tag="mask")
            nc.sync.dma_start(mask_sb[:], maskd[:])
            ones_f = small.tile([1, E], F32, tag="ones_f")
            nc.gpsimd.memset(ones_f[:], 1.0)
            ones_r = small.tile([1, E], F32R, tag="ones_r")
            nc.vector.tensor_copy(ones_r[:], ones_f[:])

            # ---- Wo weights (needed last; DMA engines idle later anyway) ----
            wo_sb = []
            for t in range(T4):
                wt = wo_pool.tile([P, D], F32R, tag="wo", name=f"wo{t}")
                nc.sync.dma_start(wt[:], wo_c[t])
                wo_sb.append(wt)

            def load_chunks(dram_c, n, tag, pool, shape):
                tiles = []
                for ic in range(n):
                    t = pool.tile(shape, F32R, tag=tag, name=f"{tag}{ic}")
                    nc.sync.dma_start(t[:], dram_c[ic])
                    tiles.append(t)
                return tiles

            qt = [qk_pool.tile([P, S], F32R, tag="qk", name=f"qt{i}")
                  for i in range(T4)]
            kt = [qk_pool.tile([P, S], F32R, tag="qk", name=f"kt{i}")
                  for i in range(T4)]

            # ---- Q^T / K^T projections ----
            for x_c, w_c, dest in ((xtq_c, wq_c, qt), (xtk_c, wk_c, kt)):
                x_sb = load_chunks(x_c, IC, "xt", xt_pool, [P, S])
                w_sb = load_chunks(w_c, IC, "wp", w_pool, [P, DL])
                for t in range(T4):
                    for sc in range(2):
                        ps = psP.tile([P, NQ], F32, tag="psP")
                        for ic in range(IC):
                            nc.tensor.matmul(
                                ps[:],
                                w_sb[ic][:, t * P:(t + 1) * P],
                                x_sb[ic][:, sc * NQ:(sc + 1) * NQ],
                                start=(ic == 0), stop=(ic == IC - 1))
                        if dve_evac:
                            nc.vector.tensor_copy(
                                dest[t][:, sc * NQ:(sc + 1) * NQ], ps[:])
                        else:
                            nc.scalar.copy(
                                dest[t][:, sc * NQ:(sc + 1) * NQ], ps[:])

            # ---- V projection -> V2 = [V*mask | mask] ----
            x_sb = load_chunks(xtv_c, IC, "xt", xt_pool, [P, S])
            w_sb = load_chunks(wv_c, IC, "wp", w_pool, [P, DL])
            v2 = []
            for skc in range(KC):
                ps = psP.tile([P, NQ], F32, tag="psP")
                for ic in range(IC):
                    nc.tensor.matmul(
                        ps[:],
                        x_sb[ic][:, skc * P:(skc + 1) * P],
                        w_sb[ic][:],
                        start=(ic == 0), stop=(ic == IC - 1))
                v2t = v2_pool.tile([P, H, E + 1], F32R, tag="v2")
                nc.vector.tensor_scalar_mul(
                    v2t[:, :, 0:E],
                    ps[:].rearrange("p (h e) -> p h e", h=H),
                    mask_sb[:, skc:skc + 1])
                nc.vector.tensor_copy(
                    v2t[:, :, E:E + 1],
                    mask_sb[:, skc:skc + 1, None].to_broadcast((P, H, 1)))
                v2.append(v2t)

            # ---- attention: 3-stage pipeline over (h, qc) iterations ----
            # stage A (iter i):   8 score MMs + 4 paired exps -> pT tiles,
            #                     interleaved with stage B of iter i-1
            # stage B (iter i-1): 8 attnV MMs -> pso ; fast reciprocal of
            #                     the denominator row
            # stage C (iter i-2): ones-matmul broadcast of 1/r ; rb evac ;
            #                     ct = numerator * rb
            ct = [ct_pool.tile([P, S], F32R, tag="ct", name=f"ct{i}")
                  for i in range(T4)]

            def emit_scores_mm(h, qc, j):
                """Emit the j-th pair of score matmuls; returns pT tile(s)."""
                t, half = h // 2, h % 2
                pb = half * E
                if paired_exp:
                    pss = psS.tile([P, 2 * NQ], F32, tag="psS")
                    for half_k in range(2):
                        kc = 2 * j + half_k
                        nc.tensor.matmul(
                            pss[:, half_k * NQ:(half_k + 1) * NQ],
                            kt[t][pb:pb + E, kc * P:(kc + 1) * P],
                            qt[t][pb:pb + E, qc * NQ:(qc + 1) * NQ],
                            start=True, stop=True,
                            tile_position=(pb, 0))
                    pt = p_pool.tile([P, 2 * NQ], F32R, tag="pt")
                    nc.scalar.activation(pt[:], pss[:], EXP, scale=0.125)
                    return [pt]
                tiles = []
                for half_k in range(2):
                    kc = 2 * j + half_k
                    pss = psS.tile([P, NQ], F32, tag="psS")
                    nc.tensor.matmul(
                        pss[:],
                        kt[t][pb:pb + E, kc * P:(kc + 1) * P],
                        qt[t][pb:pb + E, qc * NQ:(qc + 1) * NQ],
                        start=True, stop=True,
                        tile_position=(pb, 0))
                    pt = p_pool.tile([P, NQ], F32R, tag="pt")
                    nc.scalar.activation(pt[:], pss[:], EXP, scale=0.125)
                    tiles.append(pt)
                return tiles

            def emit_attnv_mm(h, p_tiles, pso, kc):
                if paired_exp:
                    rhs = p_tiles[kc // 2][0][:, (kc % 2) * NQ:(kc % 2 + 1) * NQ]
                else:
                    rhs = p_tiles[kc // 2][kc % 2][:]
                nc.tensor.matmul(
                    pso[0:E + 1, :],
                    v2[kc][:, h, :],
                    rhs,
                    start=(kc == 0), stop=(kc == KC - 1))

            def emit_recip(pso):
                # The reciprocal_approx custom-DVE ops corrupt when reading
                # PSUM at a nonzero base partition; stage the denominator row
                # into a partition-0 SBUF tile first.
                recip_f = small.tile([1, NQ], F32, tag="recip_f")
                if recip_mode in ('accurate', 'fast'):
                    denom = small.tile([1, NQ], F32, tag="denom")
                    nc.vector.tensor_copy(denom[:], pso[E:E + 1, :])
                    if recip_mode == 'accurate':
                        scratch = small.tile([1, NQ], F32, tag="scratch")
                        nc.vector.reciprocal_approx_accurate(
                            recip_f[:], denom[:], scratch[:])
                    else:
                        nc.vector.reciprocal_approx_fast(recip_f[:], denom[:])
                else:
                    nc.vector.reciprocal(recip_f[:], pso[E:E + 1, :])
                recip_r = small.tile([1, NQ], F32R, tag="recip_r")
                nc.vector.tensor_copy(recip_r[:], recip_f[:])
                return recip_r

            def emit_norm(h, qc, pso, recip_r):
                psr = psP.tile([P, NQ], F32, tag="psP")
                nc.tensor.matmul(psr[0:E, :], ones_r[:], recip_r[:],
                                 start=True, stop=True)
                rb = rb_pool.tile([E, NQ], F32, tag="rb")
                if dve_evac:
                    nc.vector.tensor_copy(rb[:], psr[0:E, :])
                else:
                    nc.scalar.copy(rb[:], psr[0:E, :])
                nc.vector.tensor_tensor(
                    ct[h // 2][(h % 2) * E:(h % 2) * E + E,
                               qc * NQ:(qc + 1) * NQ],
                    pso[0:E, :], rb[:], op=MULT)

            # NOTE: score matmuls (64-row tiling mode) must NOT be interleaved
            # inside the attnV accumulation group (128-row mode) — per-MM
            # tiling-mode switches inside an open PSUM accumulation group
            # hang/corrupt on hardware.  Keep block structure per iteration.
            iters = [(h, qc) for h in range(H) for qc in range(2)]
            stage_b = None   # (h, qc, p_tiles)
            stage_c = None   # (h, qc, pso, recip_r)
            for h, qc in iters:
                prev = stage_b
                p_tiles = [emit_scores_mm(h, qc, j) for j in range(4)]
                if prev is not None:
                    pso = psO.tile([P, NQ], F32, tag="psO")
                    for kc in range(KC):
                        emit_attnv_mm(prev[0], prev[2], pso, kc)
                    recip_r = emit_recip(pso)
                if stage_c is not None:
                    emit_norm(*stage_c)
                    stage_c = None
                if prev is not None:
                    stage_c = (prev[0], prev[1], pso, recip_r)
                stage_b = (h, qc, p_tiles)

            # drain the pipeline
            h, qc, p_tiles = stage_b
            pso = psO.tile([P, NQ], F32, tag="psO")
            for kc in range(KC):
                emit_attnv_mm(h, p_tiles, pso, kc)
            recip_r = emit_recip(pso)
            if stage_c is not None:
                emit_norm(*stage_c)
            emit_norm(h, qc, pso, recip_r)

            # ---- output projection: out[q,o] = sum_t ct[t].T @ wo[t] ----
            for qc8 in range(KC):
                for oc in range(2):
                    ps = psP.tile([P, NQ], F32, tag="psP")
                    for t in range(T4):
                        nc.tensor.matmul(
                            ps[:],
                            ct[t][:, qc8 * P:(qc8 + 1) * P],
                            wo_sb[t][:, oc * NQ:(oc + 1) * NQ],
                            start=(t == 0), stop=(t == T4 - 1))
                    osb = out_pool.tile([P, NQ], F32, tag="osb")
                    nc.vector.tensor_copy(osb[:], ps[:])
                    nc.sync.dma_start(
                        out[qc8 * P:(qc8 + 1) * P, oc * NQ:(oc + 1) * NQ],
                        osb[:])

    nc.compile()
    return nc


def make_in_maps(queries, keys, values, valid_lens, W_q, W_k, W_v, W_o):
    queries = np.asarray(queries, dtype=np.float32)
    keys = np.asarray(keys, dtype=np.float32)
    values = np.asarray(values, dtype=np.float32)
    valid_lens = np.asarray(valid_lens)
    W_q = np.asarray(W_q, dtype=np.float32)
    W_k = np.asarray(W_k, dtype=np.float32)
    W_v = np.asarray(W_v, dtype=np.float32)
    W_o = np.asarray(W_o, dtype=np.float32)

    xtq = [np.ascontiguousarray(queries[b].T) for b in range(B)]
    xtk = [np.ascontiguousarray(keys[b].T) for b in range(B)]
    xtv = [np.ascontiguousarray(values[b].T) for b in range(B)]
    wqt = [np.ascontiguousarray(W_q[hg * DL:(hg + 1) * DL, :].T) for hg in range(2)]
    wkt = [np.ascontiguousarray(W_k[hg * DL:(hg + 1) * DL, :].T) for hg in range(2)]
    wvt = [np.ascontiguousarray(W_v[hg * DL:(hg + 1) * DL, :].T) for hg in range(2)]
    wot = [np.ascontiguousarray(W_o[:, hg * DL:(hg + 1) * DL].T) for hg in range(2)]

    in_maps = []
    for c in range(N_CORES):
        b, hg = c // 2, c % 2
        L = int(valid_lens[b])
        k_idx = np.arange(S).reshape(KC, P).T  # [P, KC]
        maskd = (k_idx < L).astype(np.float32)
        in_maps.append({
            "xtq": xtq[b], "xtk": xtk[b], "xtv": xtv[b],
            "wq": wqt[hg], "wk": wkt[hg], "wv": wvt[hg], "wo": wot[hg],
            "maskd": np.ascontiguousarray(maskd),
        })
    return in_maps


def gather(results):
    out = np.empty((B, S, D), dtype=np.float32)
    for b in range(B):
        out[b] = results[2 * b]["out"] + results[2 * b + 1]["out"]
    return out


def kernel(queries, keys, values, valid_lens, W_q, W_k, W_v, W_o):
    from concourse.bass_utils import run_bass_kernel_spmd

    if "nc" not in _prog_cache:
        _prog_cache["nc"] = build_program()
    nc = _prog_cache["nc"]

    in_maps = make_in_maps(queries, keys, values, valid_lens,
                           W_q, W_k, W_v, W_o)
    res = run_bass_kernel_spmd(nc, in_maps, core_ids=list(range(N_CORES)))
    return gather(res.results)


# revision 10
# speedup vs baseline: 1.2556x; 1.1902x over previous
"""Multi-head attention (B=4, S=1024, D=1024, H=16) on 8 TRN2 NeuronCores.

Sharding: data parallel on batch (4) x tensor parallel on heads (2 groups of
8 heads).  Core c handles batch c//2 and heads (c%2)*8 .. (c%2)*8+7.

Per-core dataflow (everything in "transposed" space so no on-device
transposes are needed):
  QT [512,1024] (d_out on partitions), KT likewise, V [1024,512] natural.
  V2 [k,h,65] = [V*mask | mask]  (65th column -> masked row-sums via matmul)
  scoresT[k,q] = KT_h.T @ QT_h   (K=64 contraction)
  pT = exp(scoresT/8)            (no max subtraction; scores are O(1))
  attnV psum[0:65] = V2_h.T @ pT (rows 0:64 numerator^T, row 64 denominator)
  normalize: fast reciprocal + ones-matmul partition broadcast (3-stage
  software pipeline keeps it off the PE critical path)
  Wo: out[q,o] partial = CT.T @ WoT_loc ; host adds the two head-group halves.

Matmuls run as float32r (~1.5e-4 rel err, full PE rate at N=512).
"""
import sys

if '/opt/trn_rl_repo' not in sys.path:
    sys.path.insert(0, '/opt/trn_rl_repo')

import numpy as np

P = 128
B, S, D = 4, 1024, 1024
DL = 512          # local d_out (8 heads x 64)
H = 8             # local heads
E = 64            # head dim
IC = D // P       # 8 contraction chunks for projections
KC = S // P       # 8 key-position chunks
T4 = DL // P      # 4 tiles holding QT/KT/CT rows
NQ = 512          # matmul moving free dim
N_CORES = 8

_prog_cache = {}


def build_program(paired_exp=False, dve_evac=True, recip_mode='fast'):
    import concourse.tile as tile
    from concourse import bacc, mybir

    F32 = mybir.dt.float32
    F32R = mybir.dt.float32r
    EXP = mybir.ActivationFunctionType.Exp
    MULT = mybir.AluOpType.mult

    nc = bacc.Bacc("TRN2", target_bir_lowering=False, debug=False,
                   enable_asserts=False, num_devices=N_CORES)

    xtq = nc.dram_tensor("xtq", (D, S), F32R, kind="ExternalInput").ap()
    xtk = nc.dram_tensor("xtk", (D, S), F32R, kind="ExternalInput").ap()
    xtv = nc.dram_tensor("xtv", (D, S), F32R, kind="ExternalInput").ap()
    wq = nc.dram_tensor("wq", (D, DL), F32R, kind="ExternalInput").ap()
    wk = nc.dram_tensor("wk", (D, DL), F32R, kind="ExternalInput").ap()
    wv = nc.dram_tensor("wv", (D, DL), F32R, kind="ExternalInput").ap()
    wo = nc.dram_tensor("wo", (DL, D), F32R, kind="ExternalInput").ap()
    maskd = nc.dram_tensor("maskd", (P, KC), F32, kind="ExternalInput").ap()
    out = nc.dram_tensor("out", (S, D), F32, kind="ExternalOutput").ap()

    xtq_c = xtq.rearrange("(ic p) s -> ic p s", p=P)
    xtk_c = xtk.rearrange("(ic p) s -> ic p s", p=P)
    xtv_c = xtv.rearrange("(ic p) s -> ic p s", p=P)
    wq_c = wq.rearrange("(ic p) o -> ic p o", p=P)
    wk_c = wk.rearrange("(ic p) o -> ic p o", p=P)
    wv_c = wv.rearrange("(ic p) o -> ic p o", p=P)
    wo_c = wo.rearrange("(t p) o -> t p o", p=P)

    with tile.TileContext(nc) as tc:
        with tc.tile_pool(name="xt", bufs=11) as xt_pool, \
             tc.tile_pool(name="wp", bufs=10) as w_pool, \
             tc.tile_pool(name="wop", bufs=4) as wo_pool, \
             tc.tile_pool(name="qk", bufs=8) as qk_pool, \
             tc.tile_pool(name="v2p", bufs=8) as v2_pool, \
             tc.tile_pool(name="pp", bufs=16) as p_pool, \
             tc.tile_pool(name="ctp", bufs=4) as ct_pool, \
             tc.tile_pool(name="sm", bufs=2) as small, \
             tc.tile_pool(name="rbp", bufs=2) as rb_pool, \
             tc.tile_pool(name="ob", bufs=2) as out_pool, \
             tc.tile_pool(name="psP", bufs=2, space="PSUM") as psP, \
             tc.tile_pool(name="psS", bufs=4, space="PSUM") as psS, \
             tc.tile_pool(name="psO", bufs=2, space="PSUM") as psO:

            # ---- constants / small inputs ----
            mask_sb = small.tile([P, KC], F32, tag="mask")
            nc.sync.dma_start(mask_sb[:], maskd[:])
            ones_f = small.tile([1, E], F32, tag="ones_f")
            nc.gpsimd.memset(ones_f[:], 1.0)
            ones_r = small.tile([1, E], F32R, tag="ones_r")
            nc.vector.tensor_copy(ones_r[:], ones_f[:])

            # ---- Wo weights (needed last; DMA engines idle later anyway) ----
            wo_sb = []
            for t in range(T4):
                wt = wo_pool.tile([P, D], F32R, tag="wo", name=f"wo{t}")
                nc.sync.dma_start(wt[:], wo_c[t])
                wo_sb.append(wt)

            def load_chunks(dram_c, n, tag, pool, shape):
                tiles = []
                for ic in range(n):
                    t = pool.tile(shape, F32R, tag=tag, name=f"{tag}{ic}")
                    nc.sync.dma_start(t[:], dram_c[ic])
                    tiles.append(t)
                return tiles

            qt = [qk_pool.tile([P, S], F32R, tag="qk", name=f"qt{i}")
                  for i in range(T4)]
            kt = [qk_pool.tile([P, S], F32R, tag="qk", name=f"kt{i}")
                  for i in range(T4)]

            # ---- Q^T / K^T projections ----
            for x_c, w_c, dest in ((xtq_c, wq_c, qt), (xtk_c, wk_c, kt)):
                x_sb = load_chunks(x_c, IC, "xt", xt_pool, [P, S])
                w_sb = load_chunks(w_c, IC, "wp", w_pool, [P, DL])
                for t in range(T4):
                    for sc in range(2):
                        ps = psP.tile([P, NQ], F32, tag="psP")
                        for ic in range(IC):
                            nc.tensor.matmul(
                                ps[:],
                                w_sb[ic][:, t * P:(t + 1) * P],
                                x_sb[ic][:, sc * NQ:(sc + 1) * NQ],
                                start=(ic == 0), stop=(ic == IC - 1))
                        if dve_evac:
                            nc.vector.tensor_copy(
                                dest[t][:, sc * NQ:(sc + 1) * NQ], ps[:])
                        else:
                            nc.scalar.copy(
                                dest[t][:, sc * NQ:(sc + 1) * NQ], ps[:])

            # ---- V projection -> V2 = [V*mask | mask] ----
            x_sb = load_chunks(xtv_c, IC, "xt", xt_pool, [P, S])
            w_sb = load_chunks(wv_c, IC, "wp", w_pool, [P, DL])
            v2 = []
            for skc in range(KC):
                ps = psP.tile([P, NQ], F32, tag="psP")
                for ic in range(IC):
                    nc.tensor.matmul(
                        ps[:],
                        x_sb[ic][:, skc * P:(skc + 1) * P],
                        w_sb[ic][:],
                        start=(ic == 0), stop=(ic == IC - 1))
                v2t = v2_pool.tile([P, H, E + 1], F32R, tag="v2")
                nc.vector.tensor_scalar_mul(
                    v2t[:, :, 0:E],
                    ps[:].rearrange("p (h e) -> p h e", h=H),
                    mask_sb[:, skc:skc + 1])
                nc.vector.tensor_copy(
                    v2t[:, :, E:E + 1],
                    mask_sb[:, skc:skc + 1, None].to_broadcast((P, H, 1)))
                v2.append(v2t)

            # ---- attention: 3-stage pipeline over (h, qc) iterations ----
            # stage A (iter i):   8 score MMs + 4 paired exps -> pT tiles,
            #                     interleaved with stage B of iter i-1
            # stage B (iter i-1): 8 attnV MMs -> pso ; fast reciprocal of
            #                     the denominator row
            # stage C (iter i-2): ones-matmul broadcast of 1/r ; rb evac ;
            #                     ct = numerator * rb
            ct = [ct_pool.tile([P, S], F32R, tag="ct", name=f"ct{i}")
                  for i in range(T4)]

            def emit_scores_mm(h, qc, j):
                """Emit the j-th pair of score matmuls; returns pT tile(s)."""
                t, half = h // 2, h % 2
                pb = half * E
                if paired_exp:
                    pss = psS.tile([P, 2 * NQ], F32, tag="psS")
                    for half_k in range(2):
                        kc = 2 * j + half_k
                        nc.tensor.matmul(
                            pss[:, half_k * NQ:(half_k + 1) * NQ],
                            kt[t][pb:pb + E, kc * P:(kc + 1) * P],
                            qt[t][pb:pb + E, qc * NQ:(qc + 1) * NQ],
                            start=True, stop=True,
                            tile_position=(pb, 0))
                    pt = p_pool.tile([P, 2 * NQ], F32R, tag="pt")
                    nc.scalar.activation(pt[:], pss[:], EXP, scale=0.125)
                    return [pt]
                tiles = []
                for half_k in range(2):
                    kc = 2 * j + half_k
                    pss = psS.tile([P, NQ], F32, tag="psS")
                    nc.tensor.matmul(
                        pss[:],
                        kt[t][pb:pb + E, kc * P:(kc + 1) * P],
                        qt[t][pb:pb + E, qc * NQ:(qc + 1) * NQ],
                        start=True, stop=True,
                        tile_position=(pb, 0))
                    pt = p_pool.tile([P, NQ], F32R, tag="pt")
                    nc.scalar.activation(pt[:], pss[:], EXP, scale=0.125)
                    tiles.append(pt)
                return tiles

            def emit_attnv_mm(h, p_tiles, pso, kc):
                if paired_exp:
                    rhs = p_tiles[kc // 2][0][:, (kc % 2) * NQ:(kc % 2 + 1) * NQ]
                else:
                    rhs = p_tiles[kc // 2][kc % 2][:]
                nc.tensor.matmul(
                    pso[0:E + 1, :],
                    v2[kc][:, h, :],
                    rhs,
                    start=(kc == 0), stop=(kc == KC - 1))

            def emit_recip(pso):
                # The reciprocal_approx custom-DVE ops corrupt when reading
                # PSUM at a nonzero base partition; stage the denominator row
                # into a partition-0 SBUF tile first.
                tmp = small.tile([1, 3 * NQ], F32, tag="ntmp")
                d_ = tmp[0:1, 0:NQ]
                s_ = tmp[0:1, NQ:2 * NQ]
                r_ = tmp[0:1, 2 * NQ:3 * NQ]
                if recip_mode in ('accurate', 'fast'):
                    nc.vector.tensor_copy(d_, pso[E:E + 1, :])
                    if recip_mode == 'accurate':
                        nc.vector.reciprocal_approx_accurate(r_, d_, s_)
                    else:
                        nc.vector.reciprocal_approx_fast(r_, d_)
                else:
                    nc.vector.reciprocal(r_, pso[E:E + 1, :])
                recip_r = small.tile([1, NQ], F32R, tag="recip_r")
                nc.vector.tensor_copy(recip_r[:], r_)
                return recip_r

            def emit_norm(h, qc, pso, recip_r):
                psr = psP.tile([P, NQ], F32, tag="psP")
                nc.tensor.matmul(psr[0:E, :], ones_r[:], recip_r[:],
                                 start=True, stop=True)
                rb = rb_pool.tile([E, NQ], F32, tag="rb")
                if dve_evac:
                    nc.vector.tensor_copy(rb[:], psr[0:E, :])
                else:
                    nc.scalar.copy(rb[:], psr[0:E, :])
                nc.vector.tensor_tensor(
                    ct[h // 2][(h % 2) * E:(h % 2) * E + E,
                               qc * NQ:(qc + 1) * NQ],
                    pso[0:E, :], rb[:], op=MULT)

            # NOTE: score matmuls (64-row tiling mode) must NOT be interleaved
            # inside the attnV accumulation group (128-row mode) — per-MM
            # tiling-mode switches inside an open PSUM accumulation group
            # hang/corrupt on hardware.  Keep block structure per iteration.
            iters = [(h, qc) for h in range(H) for qc in range(2)]
            stage_b = None   # (h, qc, p_tiles)
            stage_c = None   # (h, qc, pso, recip_r)
            for h, qc in iters:
                prev = stage_b
                p_tiles = [emit_scores_mm(h, qc, j) for j in range(4)]
                if prev is not None:
                    pso = psO.tile([P, NQ], F32, tag="psO")
                    for kc in range(KC):
                        emit_attnv_mm(prev[0], prev[2], pso, kc)
                    recip_r = emit_recip(pso)
                if stage_c is not None:
                    emit_norm(*stage_c)
                    stage_c = None
                if prev is not None:
                    stage_c = (prev[0], prev[1], pso, recip_r)
                stage_b = (h, qc, p_tiles)

            # drain the pipeline
            h, qc, p_tiles = stage_b
            pso = psO.tile([P, NQ], F32, tag="psO")
            for kc in range(KC):
                emit_attnv_mm(h, p_tiles, pso, kc)
            recip_r = emit_recip(pso)
            if stage_c is not None:
                emit_norm(*stage_c)
            emit_norm(h, qc, pso, recip_r)

            # ---- output projection: out[q,o] = sum_t ct[t].T @ wo[t] ----
            for qc8 in range(KC):
                for oc in range(2):
                    ps = psP.tile([P, NQ], F32, tag="psP")
                    for t in range(T4):
                        nc.tensor.matmul(
                            ps[:],
                            ct[t][:, qc8 * P:(qc8 + 1) * P],
                            wo_sb[t][:, oc * NQ:(oc + 1) * NQ],
                            start=(t == 0), stop=(t == T4 - 1))
                    osb = out_pool.tile([P, NQ], F32, tag="osb")
                    nc.vector.tensor_copy(osb[:], ps[:])
                    nc.sync.dma_start(
                        out[qc8 * P:(qc8 + 1) * P, oc * NQ:(oc + 1) * NQ],
                        osb[:])

    nc.compile()
    return nc


def make_in_maps(queries, keys, values, valid_lens, W_q, W_k, W_v, W_o):
    queries = np.asarray(queries, dtype=np.float32)
    keys = np.asarray(keys, dtype=np.float32)
    values = np.asarray(values, dtype=np.float32)
    valid_lens = np.asarray(valid_lens)
    W_q = np.asarray(W_q, dtype=np.float32)
    W_k = np.asarray(W_k, dtype=np.float32)
    W_v = np.asarray(W_v, dtype=np.float32)
    W_o = np.asarray(W_o, dtype=np.float32)

    xtq = [np.ascontiguousarray(queries[b].T) for b in range(B)]
    xtk = [np.ascontiguousarray(keys[b].T) for b in range(B)]
    xtv = [np.ascontiguousarray(values[b].T) for b in range(B)]
    wqt = [np.ascontiguousarray(W_q[hg * DL:(hg + 1) * DL, :].T) for hg in range(2)]
    wkt = [np.ascontiguousarray(W_k[hg * DL:(hg + 1) * DL, :].T) for hg in range(2)]
    wvt = [np.ascontiguousarray(W_v[hg * DL:(hg + 1) * DL, :].T) for hg in range(2)]
    wot = [np.ascontiguousarray(W_o[:, hg * DL:(hg + 1) * DL].T) for hg in range(2)]

    in_maps = []
    for c in range(N_CORES):
        b, hg = c // 2, c % 2
        L = int(valid_lens[b])
        k_idx = np.arange(S).reshape(KC, P).T  # [P, KC]
        maskd = (k_idx < L).astype(np.float32)
        in_maps.append({
            "xtq": xtq[b], "xtk": xtk[b], "xtv": xtv[b],
            "wq": wqt[hg], "wk": wkt[hg], "wv": wvt[hg], "wo": wot[hg],
            "maskd": np.ascontiguousarray(maskd),
        })
    return in_maps


def gather(results):
    out = np.empty((B, S, D), dtype=np.float32)
    for b in range(B):
        out[b] = results[2 * b]["out"] + results[2 * b + 1]["out"]
    return out


def kernel(queries, keys, values, valid_lens, W_q, W_k, W_v, W_o):
    from concourse.bass_utils import run_bass_kernel_spmd

    if "nc" not in _prog_cache:
        _prog_cache["nc"] = build_program()
    nc = _prog_cache["nc"]

    in_maps = make_in_maps(queries, keys, values, valid_lens,
                           W_q, W_k, W_v, W_o)
    res = run_bass_kernel_spmd(nc, in_maps, core_ids=list(range(N_CORES)))
    return gather(res.results)
